# revision 9
# baseline (speedup 1.0000x reference)
"""KSparseFFTClassifier Trainium2 kernel.

Math: reference computes
    h   = x @ W_proj.T + b_proj                      (bs, 129)
    h  *= scale  (sqrt(2) on dims 1..64)
    out = IDFT65(h[:, :65]) + h[:, 65:] @ Ws.T       (bs, 16384)

The zero-padded orthonormal IDFT of the 65 nonzero frequency components is a
dense matmul against a (65, N) cos/sin basis; the DC row of that basis is the
constant 1/sqrt(N).  So with M = [cos/sin basis for h dims 1..64; Ws.T]
(128 x N):

    out[b, n] = hT[1:129, b] @ M[:, n] + (h[b, 0] + b0) / sqrt(N)

The kernel is HBM-DMA bound (the fp32 output write is 32 MiB/core), so the
read path runs in bf16 and the 64 trig rows of M are generated on device
instead of loaded:  m' = (k*n + (N/4)*is_cos) mod N,  row = Sin(2pi/N*m' - pi),
with the amplitude -sqrt(2)/sqrt(N) folded into the per-partition scale of the
mm1 eviction.  Only Ws.T (64 x N, bf16) is read from HBM for mm2.

Sharding: data-parallel over batch, 512 rows per core on 8 cores.
"""

import numpy as np

BS = 4096
IN_DIM = 2048
N = 16384
K = 32
SLACK = 64
NCORES = 8
BC = BS // NCORES        # 512 batch rows per core
P = 128
KT = IN_DIM // P         # 16 contraction tiles for matmul1
NCHUNK = 4096            # output column chunk (SBUF out tile free size)
NCH = N // NCHUNK        # 4

MM1_DT = "bfloat16"      # x / W_proj dtype for matmul1
MM2_DT = "bfloat16"      # hT / M dtype for matmul2

_NC_CACHE = {}


def _np_dt(name):
    import ml_dtypes
    return {
        "bfloat16": ml_dtypes.bfloat16,
        "float8e4": ml_dtypes.float8_e4m3,
        "float32": np.float32,
        "float32r": np.float32,
    }[name]


def _build_nc(mm1_name, mm2_name):
    import concourse.bacc as bacc
    import concourse.mybir as mybir
    import concourse.tile as tile

    f32 = mybir.dt.float32
    i32 = mybir.dt.int32
    mm1 = getattr(mybir.dt, mm1_name)
    mm2 = getattr(mybir.dt, mm2_name)
    Alu = mybir.AluOpType
    Ident = mybir.ActivationFunctionType.Identity
    Sin = mybir.ActivationFunctionType.Sin

    nc = bacc.Bacc("TRN2", target_bir_lowering=False)

    bf16 = mybir.dt.bfloat16
    xT = nc.dram_tensor("xT", [P, KT * BC], mm1, kind="ExternalInput")
    w1t = nc.dram_tensor("w1t", [P, KT * P], bf16, kind="ExternalInput")
    w0 = nc.dram_tensor("w0", [P, KT], bf16, kind="ExternalInput")
    wst = nc.dram_tensor("wst", [SLACK, N], mm2, kind="ExternalInput")
    sb2 = nc.dram_tensor("sb2", [P, 2], f32, kind="ExternalInput")   # scale, bias
    gk2 = nc.dram_tensor("gk2", [SLACK, 4], f32, kind="ExternalInput")  # k, off, -pi
    cst = nc.dram_tensor("cst", [1, 1], f32, kind="ExternalInput")
    out = nc.dram_tensor("out", [BC, N], f32, kind="ExternalOutput")

    TWO_PI_N = float(2.0 * np.pi / N)
    GC = 2048                 # trig-generation chunk (free cols)

    with tile.TileContext(nc) as tc:
        with (
            tc.tile_pool(name="wp", bufs=1) as wp,
            tc.tile_pool(name="xp", bufs=1) as xp,
            tc.tile_pool(name="mp", bufs=1) as mp,
            tc.tile_pool(name="gp", bufs=2) as gp,
            tc.tile_pool(name="hp", bufs=1) as hp,
            tc.tile_pool(name="op", bufs=3) as op,
            tc.tile_pool(name="ps", bufs=4, space="PSUM") as ps,
            tc.tile_pool(name="ps1", bufs=1, space="PSUM") as ps1,
            tc.tile_pool(name="ps2", bufs=1, space="PSUM") as ps2,
        ):
            # generation constants first so trig gen can start immediately
            gk2_sb = wp.tile([SLACK, 4], f32, tag="gk2")
            nc.sync.dma_start(out=gk2_sb[:, :], in_=gk2[:, :])

            # x transposed, packed on host as 4 groups of 4 k-tiles (first:
            # mm1 blocks on it)
            xg = []
            for g in range(4):
                t = xp.tile([P, 4 * BC], mm1, tag=f"xg{g}")
                nc.sync.dma_start(out=t[:, :], in_=xT[:, g * 4 * BC:(g + 1) * 4 * BC])
                xg.append(t)

            w1t_sb = wp.tile([P, KT * P], bf16, tag="w1t")
            nc.sync.dma_start(out=w1t_sb[:, :], in_=w1t[:, :])
            w0_sb = wp.tile([P, KT], bf16, tag="w0")
            nc.sync.dma_start(out=w0_sb[:, :], in_=w0[:, :])
            sb2_sb = wp.tile([P, 2], f32, tag="sb2")
            nc.sync.dma_start(out=sb2_sb[:, :], in_=sb2[:, :])
            cst_sb = wp.tile([1, 1], f32, tag="cst")
            nc.sync.dma_start(out=cst_sb[:, :], in_=cst[:, :])
            ones_sb = wp.tile([1, 1], f32, tag="ones")
            nc.vector.memset(ones_sb[:, :], 1.0)

            # M tiles: partitions 0..63 = generated trig rows, 64..127 = Ws.T
            # trig row p (k=p//2+1, even=cos): m' = ((k*n + off) & (N-1)) ^ N/2
            # then Sin(2pi/N*m' - pi) == cos/sin(2pi*k*n/N) exactly (xor flips
            # the halves so the -pi recenters into the table domain).
            mm = []
            for ti in range(NCH):
                m = mp.tile([P, NCHUNK], mm2, tag=f"m{ti}")
                nc.sync.dma_start(
                    out=m[64:128, :], in_=wst[:, ti * NCHUNK:(ti + 1) * NCHUNK]
                )
                for c in range(NCHUNK // GC):
                    it = gp.tile([SLACK, GC], i32, tag="it")
                    nc.gpsimd.iota(
                        it[:, :], pattern=[[1, GC]], base=ti * NCHUNK + c * GC,
                        channel_multiplier=0,
                    )
                    ph = gp.tile([SLACK, GC], i32, tag="ph")
                    nc.vector.tensor_scalar(
                        ph[:, :], it[:, :], gk2_sb[:, 0:1], gk2_sb[:, 1:2],
                        Alu.mult, Alu.add,
                    )
                    md = gp.tile([SLACK, GC], i32, tag="md")
                    nc.vector.tensor_scalar(
                        md[:, :], ph[:, :], int(N - 1), int(N // 2),
                        Alu.bitwise_and, Alu.bitwise_xor,
                    )
                    nc.scalar.activation(
                        m[0:64, c * GC:(c + 1) * GC], md[:, :], Sin,
                        bias=gk2_sb[:, 2:3], scale=TWO_PI_N,
                    )
                mm.append(m)

            # matmul1: hT[d, b] for d = h dims 1..128
            hT_ps = ps1.tile([P, BC], f32, tag="hT")
            for kt in range(KT):
                nc.tensor.matmul(
                    hT_ps[:, :],
                    lhsT=w1t_sb[:, kt * P:(kt + 1) * P],
                    rhs=xg[kt // 4][:, (kt % 4) * BC:(kt % 4 + 1) * BC],
                    start=(kt == 0),
                    stop=(kt == KT - 1),
                )
            # evict with per-partition scale fold: hT = (h + b) * t_p
            # (t_p = -sqrt2/sqrtN on trig rows, 1 on slack rows)
            hT_sb = hp.tile([P, BC], bf16, tag="hT_sb")
            nc.vector.tensor_scalar(
                hT_sb[:, :], hT_ps[:, :], sb2_sb[:, 0:1], sb2_sb[:, 1:2],
                Alu.mult, Alu.add,
            )

            # dc row: h dim 0 (as (1, BC)), then PE-transpose to (P, 4)
            dcr_ps = ps2.tile([1, BC], f32, tag="dcr")
            for kt in range(KT):
                nc.tensor.matmul(
                    dcr_ps[:, :],
                    lhsT=w0_sb[:, kt:kt + 1],
                    rhs=xg[kt // 4][:, (kt % 4) * BC:(kt % 4 + 1) * BC],
                    start=(kt == 0),
                    stop=(kt == KT - 1),
                )
            dcr_sb = hp.tile([1, BC], f32, tag="dcr_sb")
            nc.scalar.activation(
                dcr_sb[:, :], dcr_ps[:, :], Ident,
                bias=cst_sb[0:1, 0:1], scale=float(1.0 / np.sqrt(N)),
            )
            dc_sb = hp.tile([P, BC // P], f32, tag="dc_sb")
            for j in range(BC // P):
                dcc_ps = ps2.tile([P, 1], f32, tag="dcc")
                nc.tensor.matmul(
                    dcc_ps[:, :],
                    lhsT=dcr_sb[0:1, j * P:(j + 1) * P],
                    rhs=ones_sb[0:1, 0:1],
                    start=True,
                    stop=True,
                )
                nc.scalar.copy(dc_sb[:, j:j + 1], dcc_ps[:, :])

            # matmul2 + DC bias-add eviction + store
            ev = 0
            for ti in range(NCH):
                for j in range(BC // P):
                    ob = op.tile([P, NCHUNK], f32, tag="ob")
                    for s in range(NCHUNK // 512):
                        pt = ps.tile([P, 512], f32, tag="mm2")
                        nc.tensor.matmul(
                            pt[:, :],
                            lhsT=hT_sb[:, j * P:(j + 1) * P],
                            rhs=mm[ti][:, s * 512:(s + 1) * 512],
                            start=True,
                            stop=True,
                        )
                        dst = ob[:, s * 512:(s + 1) * 512]
                        # 9:7 ACT:DVE split (ACT also does the 4 Sin tiles,
                        # DVE the 4 mod tiles; measured ~equal eviction rates)
                        if (ev % 16) in (1, 3, 5, 7, 9, 11, 13):
                            nc.vector.tensor_scalar_add(dst, pt[:, :], dc_sb[:, j:j + 1])
                        else:
                            nc.scalar.add(dst, pt[:, :], dc_sb[:, j:j + 1])
                        ev += 1
                    nc.sync.dma_start(
                        out=out[j * P:(j + 1) * P, ti * NCHUNK:(ti + 1) * NCHUNK],
                        in_=ob[:, :],
                    )
    nc.compile()
    return nc


def _get_nc():
    key = (MM1_DT, MM2_DT)
    if key not in _NC_CACHE:
        _NC_CACHE[key] = _build_nc(*key)
    return _NC_CACHE[key]


def _host_pack(x, W_proj, b_proj, Ws):
    import ml_dtypes
    dt1 = _np_dt(MM1_DT)
    dt2 = _np_dt(MM2_DT)
    dtw = ml_dtypes.bfloat16
    SQRT2 = np.float64(np.sqrt(np.float32(2.0)))
    isqn = 1.0 / np.sqrt(np.float64(N))

    wst = np.ascontiguousarray(Ws.T.astype(dt2))                  # (64, N)

    w1 = W_proj[1:P + 1]                                          # (128, 2048)
    w1t = np.ascontiguousarray(
        w1.T.reshape(KT, P, P).transpose(1, 0, 2).reshape(P, KT * P).astype(dtw)
    )
    w0 = np.ascontiguousarray(W_proj[0].reshape(KT, P).T.astype(dtw))  # (128, 16)

    # per-partition eviction fold: hT = (h + b) * t;  t = sqrt2/sqrtN on the
    # 64 trig rows (the generated rows have unit amplitude), 1 on slack
    t = np.ones((P,), np.float64)
    t[:2 * K] = SQRT2 * isqn
    b = b_proj[1:P + 1].astype(np.float64)
    sb2 = np.ascontiguousarray(
        np.stack([t, b * t], axis=1).astype(np.float32)           # (128, 2)
    )

    # trig-generation constants: k_p = p//2+1, off_p = (N/4) on cos (even)
    # rows, and the Sin-activation bias -pi
    kp = (np.arange(SLACK) // 2 + 1).astype(np.float32)
    off = np.where(np.arange(SLACK) % 2 == 0, N // 4, 0).astype(np.float32)
    npi = np.full(SLACK, -np.pi, np.float32)
    gk2 = np.ascontiguousarray(
        np.stack([kp, off, npi, np.zeros(SLACK, np.float32)], axis=1)  # (64, 4)
    )

    cstv = np.asarray(b_proj[0] * isqn, np.float32).reshape(1, 1)

    xts = []
    for c in range(NCORES):
        xc = x[c * BC:(c + 1) * BC]                               # (512, 2048)
        xt = np.ascontiguousarray(
            xc.T.reshape(KT, P, BC).transpose(1, 0, 2).reshape(P, KT * BC).astype(dt1)
        )
        xts.append(xt)
    return wst, w1t, w0, sb2, gk2, cstv, xts


def kernel(x, W_proj, b_proj, Ws, _trace=False, _tmpdir=None):
    from concourse import bass_utils

    x = np.ascontiguousarray(x, np.float32)
    W_proj = np.ascontiguousarray(W_proj, np.float32)
    b_proj = np.ascontiguousarray(b_proj, np.float32)
    Ws = np.ascontiguousarray(Ws, np.float32)

    wst, w1t, w0, sb2, gk2, cstv, xts = _host_pack(x, W_proj, b_proj, Ws)
    nc = _get_nc()

    in_maps = [
        {"xT": xts[c], "w1t": w1t, "w0": w0, "wst": wst, "sb2": sb2,
         "gk2": gk2, "cst": cstv}
        for c in range(NCORES)
    ]
    kw = {}
    if _trace:
        kw = dict(trace=True, tmpdir=_tmpdir, trace_cores=[0])
    res = bass_utils.run_bass_kernel_spmd(nc, in_maps, core_ids=list(range(NCORES)), **kw)
    out = np.concatenate([r["out"] for r in res.results], axis=0)
    if _trace:
        return out, res
    return out


# revision 14
# speedup vs baseline: 1.2244x; 1.2244x over previous
"""KSparseFFTClassifier Trainium2 kernel.

Math: reference computes
    h   = x @ W_proj.T + b_proj                      (bs, 129)
    h  *= scale  (sqrt(2) on dims 1..64)
    out = IDFT65(h[:, :65]) + h[:, 65:] @ Ws.T       (bs, 16384)

The zero-padded orthonormal IDFT of the 65 nonzero frequency components is a
dense matmul against a (65, N) cos/sin basis; the DC row of that basis is the
constant 1/sqrt(N).  So with M = [cos/sin basis for h dims 1..64; Ws.T]
(128 x N):

    out[b, n] = hT[1:129, b] @ M[:, n] + (h[b, 0] + b0) / sqrt(N)

The kernel is HBM-DMA bound (the fp32 output write is 32 MiB/core), so the
read path runs in bf16 and the 64 trig rows of M are generated on device
instead of loaded:  m' = (k*n + (N/4)*is_cos) mod N,  row = Sin(2pi/N*m' - pi),
with the amplitude -sqrt(2)/sqrt(N) folded into the per-partition scale of the
mm1 eviction.  Only Ws.T (64 x N, bf16) is read from HBM for mm2.

Sharding: data-parallel over batch, 512 rows per core on 8 cores.
"""

import numpy as np

BS = 4096
IN_DIM = 2048
N = 16384
K = 32
SLACK = 64
NCORES = 8
BC = BS // NCORES        # 512 batch rows per core
P = 128
KT = IN_DIM // P         # 16 contraction tiles for matmul1
NCHUNK = 4096            # output column chunk (SBUF out tile free size)
NCH = N // NCHUNK        # 4

MM1_DT = "bfloat16"      # x / W_proj dtype for matmul1
MM2_DT = "bfloat16"      # hT / M dtype for matmul2

_NC_CACHE = {}


def _np_dt(name):
    import ml_dtypes
    return {
        "bfloat16": ml_dtypes.bfloat16,
        "float8e4": ml_dtypes.float8_e4m3,
        "float32": np.float32,
        "float32r": np.float32,
    }[name]


def _build_nc(mm1_name, mm2_name):
    import concourse.bacc as bacc
    import concourse.mybir as mybir
    import concourse.tile as tile

    f32 = mybir.dt.float32
    i32 = mybir.dt.int32
    mm1 = getattr(mybir.dt, mm1_name)
    mm2 = getattr(mybir.dt, mm2_name)
    Alu = mybir.AluOpType
    Ident = mybir.ActivationFunctionType.Identity
    Sin = mybir.ActivationFunctionType.Sin

    nc = bacc.Bacc("TRN2", target_bir_lowering=False)

    bf16 = mybir.dt.bfloat16
    xT = nc.dram_tensor("xT", [P, KT * BC], mm1, kind="ExternalInput")
    w1t = nc.dram_tensor("w1t", [P, KT * P], bf16, kind="ExternalInput")
    w0 = nc.dram_tensor("w0", [P, KT], bf16, kind="ExternalInput")
    wst = nc.dram_tensor("wst", [SLACK, N], mm2, kind="ExternalInput")
    sb2 = nc.dram_tensor("sb2", [P, 2], f32, kind="ExternalInput")   # scale, bias
    # gen constants: col 0 = k_p, col 1 = -pi, cols 2.. = per-chunk
    # (k_p*n0 + off_p) mod N
    GC = 2048                 # trig-generation chunk (free cols)
    NGC = N // GC
    gk2 = nc.dram_tensor("gk2", [SLACK, 2 + NGC], f32, kind="ExternalInput")
    cst = nc.dram_tensor("cst", [1, 1], f32, kind="ExternalInput")
    out = nc.dram_tensor("out", [BC, N], f32, kind="ExternalOutput")

    TWO_PI_N = float(2.0 * np.pi / N)

    with tile.TileContext(nc) as tc:
        with (
            tc.tile_pool(name="wp", bufs=1) as wp,
            tc.tile_pool(name="xp", bufs=1) as xp,
            tc.tile_pool(name="mp", bufs=1) as mp,
            tc.tile_pool(name="gp", bufs=2) as gp,
            tc.tile_pool(name="hp", bufs=1) as hp,
            tc.tile_pool(name="op", bufs=3) as op,
            tc.tile_pool(name="ps", bufs=4, space="PSUM") as ps,
            tc.tile_pool(name="ps1", bufs=1, space="PSUM") as ps1,
            tc.tile_pool(name="ps2", bufs=1, space="PSUM") as ps2,
        ):
            # generation constants first so trig gen can start immediately
            gk2_sb = wp.tile([SLACK, 2 + NGC], f32, tag="gk2")
            nc.sync.dma_start(out=gk2_sb[:, :], in_=gk2[:, :])

            # x transposed, packed on host as 4 groups of 4 k-tiles (first:
            # mm1 blocks on it)
            xg = []
            for g in range(4):
                t = xp.tile([P, 4 * BC], mm1, tag=f"xg{g}")
                nc.sync.dma_start(out=t[:, :], in_=xT[:, g * 4 * BC:(g + 1) * 4 * BC])
                xg.append(t)

            w1t_sb = wp.tile([P, KT * P], bf16, tag="w1t")
            nc.sync.dma_start(out=w1t_sb[:, :], in_=w1t[:, :])
            w0_sb = wp.tile([P, KT], bf16, tag="w0")
            nc.sync.dma_start(out=w0_sb[:, :], in_=w0[:, :])
            sb2_sb = wp.tile([P, 2], f32, tag="sb2")
            nc.sync.dma_start(out=sb2_sb[:, :], in_=sb2[:, :])
            cst_sb = wp.tile([1, 1], f32, tag="cst")
            nc.sync.dma_start(out=cst_sb[:, :], in_=cst[:, :])
            ones_sb = wp.tile([1, 1], f32, tag="ones")
            nc.vector.memset(ones_sb[:, :], 1.0)

            # M tiles: partitions 0..63 = generated trig rows, 64..127 = Ws.T
            # trig row p (k=p//2+1, even=cos):
            #   m' = ((k_p*j + c2[p, chunk]) & (N-1)) ^ N/2,
            #   row = Sin(2pi/N*m' - pi) == cos/sin(2pi*k*n/N) exactly (the
            #   xor swaps halves; -pi recenters into the Sin table domain).
            # k_p*j is chunk-independent, computed once into kn.
            mm = [
                mp.tile([P, NCHUNK], mm2, tag=f"m{ti}", name=f"m{ti}")
                for ti in range(NCH)
            ]

            it_sb = wp.tile([SLACK, GC], i32, tag="it")
            nc.gpsimd.iota(it_sb[:, :], pattern=[[1, GC]], base=0,
                           channel_multiplier=0)
            kn_sb = wp.tile([SLACK, GC], i32, tag="kn")
            nc.vector.tensor_scalar(
                kn_sb[:, :], it_sb[:, :], gk2_sb[:, 0:1], None, Alu.mult,
            )

            def load_m(ti):
                nc.sync.dma_start(
                    out=mm[ti][64:128, :],
                    in_=wst[:, ti * NCHUNK:(ti + 1) * NCHUNK],
                )

            def gen_m(ti, c):
                gc = ti * (NCHUNK // GC) + c
                ph = gp.tile([SLACK, GC], i32, tag="ph")
                nc.vector.tensor_scalar(
                    ph[:, :], kn_sb[:, :], gk2_sb[:, 2 + gc:3 + gc], None,
                    Alu.add,
                )
                md = gp.tile([SLACK, GC], i32, tag="md")
                nc.vector.tensor_scalar(
                    md[:, :], ph[:, :], int(N - 1), int(N // 2),
                    Alu.bitwise_and, Alu.bitwise_xor,
                )
                nc.scalar.activation(
                    mm[ti][0:64, c * GC:(c + 1) * GC], md[:, :], Sin,
                    bias=gk2_sb[:, 1:2], scale=TWO_PI_N,
                )

            load_m(0)
            for c in range(NCHUNK // GC):
                gen_m(0, c)

            # matmul1: hT[d, b] for d = h dims 1..128
            hT_ps = ps1.tile([P, BC], f32, tag="hT")
            for kt in range(KT):
                nc.tensor.matmul(
                    hT_ps[:, :],
                    lhsT=w1t_sb[:, kt * P:(kt + 1) * P],
                    rhs=xg[kt // 4][:, (kt % 4) * BC:(kt % 4 + 1) * BC],
                    start=(kt == 0),
                    stop=(kt == KT - 1),
                )
            # evict with per-partition scale fold: hT = (h + b) * t_p
            # (t_p = -sqrt2/sqrtN on trig rows, 1 on slack rows)
            hT_sb = hp.tile([P, BC], bf16, tag="hT_sb")
            nc.vector.tensor_scalar(
                hT_sb[:, :], hT_ps[:, :], sb2_sb[:, 0:1], sb2_sb[:, 1:2],
                Alu.mult, Alu.add,
            )

            # dc row: h dim 0 (as (1, BC)), then PE-transpose to (P, 4)
            dcr_ps = ps2.tile([1, BC], f32, tag="dcr")
            for kt in range(KT):
                nc.tensor.matmul(
                    dcr_ps[:, :],
                    lhsT=w0_sb[:, kt:kt + 1],
                    rhs=xg[kt // 4][:, (kt % 4) * BC:(kt % 4 + 1) * BC],
                    start=(kt == 0),
                    stop=(kt == KT - 1),
                )
            dcr_sb = hp.tile([1, BC], f32, tag="dcr_sb")
            nc.scalar.activation(
                dcr_sb[:, :], dcr_ps[:, :], Ident,
                bias=cst_sb[0:1, 0:1], scale=float(1.0 / np.sqrt(N)),
            )
            dc_sb = hp.tile([P, BC // P], f32, tag="dc_sb")
            for j in range(BC // P):
                dcc_ps = ps2.tile([P, 1], f32, tag="dcc")
                nc.tensor.matmul(
                    dcc_ps[:, :],
                    lhsT=dcr_sb[0:1, j * P:(j + 1) * P],
                    rhs=ones_sb[0:1, 0:1],
                    start=True,
                    stop=True,
                )
                nc.scalar.copy(dc_sb[:, j:j + 1], dcc_ps[:, :])

            # matmul2 + DC bias-add eviction + store.  The next M tile's Ws.T
            # load and trig generation are interleaved between j-blocks so
            # each engine's in-order stream never stalls the output pipeline.
            ev = 0
            for ti in range(NCH):
                for j in range(BC // P):
                    ob = op.tile([P, NCHUNK], f32, tag="ob")
                    for s in range(NCHUNK // 512):
                        pt = ps.tile([P, 512], f32, tag="mm2")
                        nc.tensor.matmul(
                            pt[:, :],
                            lhsT=hT_sb[:, j * P:(j + 1) * P],
                            rhs=mm[ti][:, s * 512:(s + 1) * 512],
                            start=True,
                            stop=True,
                        )
                        dst = ob[:, s * 512:(s + 1) * 512]
                        # ~equal DVE:ACT split (both also carry gen work)
                        if (ev % 16) in (1, 3, 5, 7, 9, 11, 13):
                            nc.vector.tensor_scalar_add(dst, pt[:, :], dc_sb[:, j:j + 1])
                        else:
                            nc.scalar.add(dst, pt[:, :], dc_sb[:, j:j + 1])
                        ev += 1
                    nc.sync.dma_start(
                        out=out[j * P:(j + 1) * P, ti * NCHUNK:(ti + 1) * NCHUNK],
                        in_=ob[:, :],
                    )
                    if ti + 1 < NCH:
                        if j == 0:
                            load_m(ti + 1)
                        elif j in (1, 2):
                            gen_m(ti + 1, j - 1)
    nc.compile()
    return nc


def _get_nc():
    key = (MM1_DT, MM2_DT)
    if key not in _NC_CACHE:
        _NC_CACHE[key] = _build_nc(*key)
    return _NC_CACHE[key]


def _host_pack(x, W_proj, b_proj, Ws):
    import ml_dtypes
    dt1 = _np_dt(MM1_DT)
    dt2 = _np_dt(MM2_DT)
    dtw = ml_dtypes.bfloat16
    SQRT2 = np.float64(np.sqrt(np.float32(2.0)))
    isqn = 1.0 / np.sqrt(np.float64(N))

    wst = np.ascontiguousarray(Ws.T.astype(dt2))                  # (64, N)

    w1 = W_proj[1:P + 1]                                          # (128, 2048)
    w1t = np.ascontiguousarray(
        w1.T.reshape(KT, P, P).transpose(1, 0, 2).reshape(P, KT * P).astype(dtw)
    )
    w0 = np.ascontiguousarray(W_proj[0].reshape(KT, P).T.astype(dtw))  # (128, 16)

    # per-partition eviction fold: hT = (h + b) * t;  t = sqrt2/sqrtN on the
    # 64 trig rows (the generated rows have unit amplitude), 1 on slack
    t = np.ones((P,), np.float64)
    t[:2 * K] = SQRT2 * isqn
    b = b_proj[1:P + 1].astype(np.float64)
    sb2 = np.ascontiguousarray(
        np.stack([t, b * t], axis=1).astype(np.float32)           # (128, 2)
    )

    # trig-generation constants: col 0 = k_p = p//2+1, col 1 = -pi, cols 2..
    # = per-chunk base (k_p*n0 + off_p) mod N, off_p = N/4 on cos (even) rows
    GC = 2048
    kp = (np.arange(SLACK) // 2 + 1).astype(np.int64)
    off = np.where(np.arange(SLACK) % 2 == 0, N // 4, 0).astype(np.int64)
    n0 = (np.arange(N // GC) * GC).astype(np.int64)
    c2 = (kp[:, None] * n0[None, :] + off[:, None]) % N            # (64, 8)
    gk2 = np.ascontiguousarray(np.concatenate([
        kp[:, None].astype(np.float32),
        np.full((SLACK, 1), -np.pi, np.float32),
        c2.astype(np.float32),
    ], axis=1))                                                    # (64, 10)

    cstv = np.asarray(b_proj[0] * isqn, np.float32).reshape(1, 1)

    xts = []
    for c in range(NCORES):
        xc = x[c * BC:(c + 1) * BC]                               # (512, 2048)
        xt = np.ascontiguousarray(
            xc.T.reshape(KT, P, BC).transpose(1, 0, 2).reshape(P, KT * BC).astype(dt1)
        )
        xts.append(xt)
    return wst, w1t, w0, sb2, gk2, cstv, xts


def kernel(x, W_proj, b_proj, Ws, _trace=False, _tmpdir=None):
    from concourse import bass_utils

    x = np.ascontiguousarray(x, np.float32)
    W_proj = np.ascontiguousarray(W_proj, np.float32)
    b_proj = np.ascontiguousarray(b_proj, np.float32)
    Ws = np.ascontiguousarray(Ws, np.float32)

    wst, w1t, w0, sb2, gk2, cstv, xts = _host_pack(x, W_proj, b_proj, Ws)
    nc = _get_nc()

    in_maps = [
        {"xT": xts[c], "w1t": w1t, "w0": w0, "wst": wst, "sb2": sb2,
         "gk2": gk2, "cst": cstv}
        for c in range(NCORES)
    ]
    kw = {}
    if _trace:
        kw = dict(trace=True, tmpdir=_tmpdir, trace_cores=[0])
    res = bass_utils.run_bass_kernel_spmd(nc, in_maps, core_ids=list(range(NCORES)), **kw)
    out = np.concatenate([r["out"] for r in res.results], axis=0)
    if _trace:
        return out, res
    return out
